# revision 4
# baseline (speedup 1.0000x reference)
"""LoRA layer kernel for Trainium2, SPMD across 8 NeuronCores.

Computes: out[b,s,h,d] = x[b,s,:] @ W_orig[:,h,d] + SCALE * (x @ A) @ B[:,h,d]

Strategy (per sharding hint, data-parallel branch):
  - Fold LoRA into the weights ON DEVICE: W_eff = W + (SCALE*A) @ B
    (associativity of matmul makes this exact up to fp rounding, and it
    turns the whole problem into one dense matmul).
  - Shard x over tokens (B*S = 8192 -> 1024 tokens per core); W/A/B replicated.
  - Per core: out_slice[1024, 2048] = xT_slice.T @ W_eff, accumulated over
    16 K-tiles of 128 into 4 PSUM banks of [128, 512].
  - Matmuls run in float32r mode (fp32 bits, FP22 multiply, fp32 accumulate):
    4x faster than true fp32 on the PE at ~1e-4 relative error.

x is fed pre-transposed ([H, tokens] per core) so the contraction dim lands on
SBUF partitions; host-side layout prep only, all FLOPs happen on device.
"""

import numpy as np

# Problem shapes (hardcoded per contract - kernel.py must be self-contained)
B, S, H = 4, 2048, 2048
NH, HD = 16, 128
N = NH * HD            # 2048 output features
RANK = 4
ALPHA = 4.0
SCALE = ALPHA / RANK   # 1.0
NCORES = 8
TOK = B * S            # 8192 tokens total
TPC = TOK // NCORES    # 1024 tokens per core

P = 128                # SBUF partitions
KT = H // P            # 16 contraction tiles
TT = TPC // P          # 8 token tiles per core
CH = 512               # psum chunk width (fp32 moving-operand / bank limit)
NCH = N // CH          # 4 chunks

_CACHE = {}


def _build_program():
    import concourse.mybir as mybir
    import concourse.tile as tile
    from concourse import bacc

    f32 = mybir.dt.float32
    f32r = mybir.dt.float32r

    nc = bacc.Bacc(None, target_bir_lowering=False, debug=False)

    # Tensors that feed matmuls directly are declared float32r (same 4-byte
    # layout as fp32; the PE truncates to FP22 on read). W goes through a
    # DVE add (the LoRA fold), which converts fp32 -> f32r on its output.
    xt = nc.dram_tensor("xt", [H, TPC], f32r, kind="ExternalInput")
    w = nc.dram_tensor("w", [H, N], f32, kind="ExternalInput")
    at = nc.dram_tensor("at", [RANK, H], f32r, kind="ExternalInput")
    bk = nc.dram_tensor("bk", [RANK, N], f32r, kind="ExternalInput")
    out = nc.dram_tensor("out", [TPC, N], f32, kind="ExternalOutput")

    with tile.TileContext(nc) as tc:
        with (
            tc.tile_pool(name="wpool", bufs=1) as wpool,
            tc.tile_pool(name="wraw", bufs=3) as wraw,
            tc.tile_pool(name="xpool", bufs=3) as xpool,
            tc.tile_pool(name="opool", bufs=2) as opool,
            tc.tile_pool(name="cpool", bufs=1) as cpool,
            tc.tile_pool(name="apool", bufs=2) as apool,
            tc.tile_pool(name="psum", bufs=8, space="PSUM") as psum,
        ):
            # LoRA B matrix, resident: [RANK, N]
            bk_t = cpool.tile([RANK, N], f32r)
            nc.sync.dma_start(bk_t[:], bk[:])

            # Prefetch first token tiles of x while W streams in.
            # Each x tile holds a full [H, 128-token] slab as [p, k, t].
            x_tiles = {}

            def load_x(t):
                x3 = xpool.tile([P, KT, P], f32r, tag="x", name=f"x3_{t}")
                src = xt[:, t * P:(t + 1) * P].rearrange("(k p) t -> p k t", p=P)
                nc.sync.dma_start(x3[:], src)
                x_tiles[t] = x3

            load_x(0)
            load_x(1)

            # Phase 1: stream W in, fold LoRA: W_eff[k] = W[k] + A_k @ B.
            # The DVE add reads raw W (fp32) + lora product (PSUM fp32) and
            # writes the resident W_eff tile as f32r (rounds on output).
            w_tiles = []
            for k in range(KT):
                wr = wraw.tile([P, N], f32, tag="wr", name=f"wr_{k}")
                nc.sync.dma_start(wr[:], w[k * P:(k + 1) * P, :])
                wt = wpool.tile([P, N], f32r, tag=f"w{k}", name=f"weff_{k}")
                at_t = apool.tile([RANK, P], f32r, tag="at", name=f"at_{k}")
                nc.sync.dma_start(at_t[:], at[:, k * P:(k + 1) * P])
                for c in range(NCH):
                    ps = psum.tile([P, CH], f32, tag="ps", name=f"psl_{k}_{c}")
                    nc.tensor.matmul(
                        ps[:],
                        at_t[:],
                        bk_t[:, c * CH:(c + 1) * CH],
                        start=True, stop=True,
                    )
                    nc.vector.tensor_add(
                        wt[:, c * CH:(c + 1) * CH],
                        wr[:, c * CH:(c + 1) * CH],
                        ps[:],
                    )
                w_tiles.append(wt)

            # Phase 2: main matmul, token tile by token tile
            for t in range(TT):
                x3 = x_tiles.pop(t)
                if t + 2 < TT:
                    load_x(t + 2)
                pss = [
                    psum.tile([P, CH], f32, tag="ps", name=f"ps_{t}_{c}")
                    for c in range(NCH)
                ]
                for k in range(KT):
                    lhsT = x3[:, k, :]
                    for c in range(NCH):
                        nc.tensor.matmul(
                            pss[c][:],
                            lhsT,
                            w_tiles[k][:, c * CH:(c + 1) * CH],
                            start=(k == 0), stop=(k == KT - 1),
                        )
                ot = opool.tile([P, N], f32, tag="o", name=f"o_{t}")
                for c in range(NCH):
                    nc.vector.tensor_copy(ot[:, c * CH:(c + 1) * CH], pss[c][:])
                nc.sync.dma_start(out[t * P:(t + 1) * P, :], ot[:])

    nc.compile()
    return nc


def _prep_inputs(x, W_orig, A_kernel, B_kernel):
    x = np.asarray(x, dtype=np.float32)
    W_orig = np.asarray(W_orig, dtype=np.float32)
    A_kernel = np.asarray(A_kernel, dtype=np.float32)
    B_kernel = np.asarray(B_kernel, dtype=np.float32)

    xT = np.ascontiguousarray(x.reshape(TOK, H).T)          # [H, TOK]
    w2d = np.ascontiguousarray(W_orig.reshape(H, N))        # [H, N]
    at = np.ascontiguousarray(A_kernel.T) * np.float32(SCALE)  # [RANK, H]
    bk = np.ascontiguousarray(B_kernel.reshape(RANK, N))    # [RANK, N]

    in_maps = []
    for i in range(NCORES):
        in_maps.append({
            "xt": np.ascontiguousarray(xT[:, i * TPC:(i + 1) * TPC]),
            "w": w2d,
            "at": at,
            "bk": bk,
        })
    return in_maps


def kernel(x, W_orig, A_kernel, B_kernel):
    from concourse.bass_utils import run_bass_kernel_spmd

    if "nc" not in _CACHE:
        _CACHE["nc"] = _build_program()
    nc = _CACHE["nc"]

    in_maps = _prep_inputs(x, W_orig, A_kernel, B_kernel)
    res = run_bass_kernel_spmd(nc, in_maps, list(range(NCORES)))
    parts = [res.results[i]["out"] for i in range(NCORES)]
    full = np.concatenate(parts, axis=0)                    # [TOK, N]
    return full.reshape(B, S, NH, HD)


# revision 6
# speedup vs baseline: 88.2334x; 88.2334x over previous
"""LoRA layer kernel for Trainium2, SPMD across 8 NeuronCores.

Computes: out[b,s,h,d] = x[b,s,:] @ W_orig[:,h,d] + SCALE * (x @ A) @ B[:,h,d]

Strategy (per sharding hint, data-parallel branch):
  - Fold LoRA into the weights ON DEVICE: W_eff = W + (SCALE*A) @ B
    (associativity of matmul makes this exact up to fp rounding, and it
    turns the whole problem into one dense matmul).
  - Shard x over tokens (B*S = 8192 -> 1024 tokens per core); W/A/B replicated.
  - Per core: out_slice[1024, 2048] = xT_slice.T @ W_eff, accumulated over
    16 K-tiles of 128 into 4 PSUM banks of [128, 512].
  - Matmuls run in float32r mode (fp32 bits, FP22 multiply, fp32 accumulate):
    4x faster than true fp32 on the PE at ~1e-4 relative error.

x is fed pre-transposed ([H, tokens] per core) so the contraction dim lands on
SBUF partitions; host-side layout prep only, all FLOPs happen on device.
"""

import numpy as np

# Problem shapes (hardcoded per contract - kernel.py must be self-contained)
B, S, H = 4, 2048, 2048
NH, HD = 16, 128
N = NH * HD            # 2048 output features
RANK = 4
ALPHA = 4.0
SCALE = ALPHA / RANK   # 1.0
NCORES = 8
TOK = B * S            # 8192 tokens total
TPC = TOK // NCORES    # 1024 tokens per core

P = 128                # SBUF partitions
KT = H // P            # 16 contraction tiles
TT = TPC // P          # 8 token tiles per core
CH = 512               # psum chunk width (fp32 moving-operand / bank limit)
NCH = N // CH          # 4 chunks

_CACHE = {}


def _build_program(reps=1):
    """Build the SPMD program. reps>1 repeats the whole body back-to-back
    (used only for timing: wall(R) - wall(1) cancels host/tunnel overhead)."""
    import concourse.mybir as mybir
    import concourse.tile as tile
    from concourse import bacc

    f32 = mybir.dt.float32
    f32r = mybir.dt.float32r

    nc = bacc.Bacc(None, target_bir_lowering=False, debug=False)

    # Tensors that feed matmuls directly are declared float32r (same 4-byte
    # layout as fp32; the PE truncates to FP22 on read). W goes through a
    # DVE add (the LoRA fold), which converts fp32 -> f32r on its output.
    xt = nc.dram_tensor("xt", [H, TPC], f32r, kind="ExternalInput")
    w = nc.dram_tensor("w", [H, N], f32, kind="ExternalInput")
    at = nc.dram_tensor("at", [RANK, H], f32r, kind="ExternalInput")
    bk = nc.dram_tensor("bk", [RANK, N], f32r, kind="ExternalInput")
    out = nc.dram_tensor("out", [TPC, N], f32, kind="ExternalOutput")

    with tile.TileContext(nc) as tc:
        with (
            tc.tile_pool(name="wpool", bufs=1) as wpool,
            tc.tile_pool(name="wraw", bufs=3) as wraw,
            tc.tile_pool(name="xpool", bufs=3) as xpool,
            tc.tile_pool(name="opool", bufs=2) as opool,
            tc.tile_pool(name="cpool", bufs=1) as cpool,
            tc.tile_pool(name="apool", bufs=2) as apool,
            tc.tile_pool(name="psum", bufs=8, space="PSUM") as psum,
        ):
            for r in range(reps):
                # LoRA B matrix, resident: [RANK, N]
                bk_t = cpool.tile([RANK, N], f32r, tag="bk", name=f"bk_{r}")
                nc.sync.dma_start(bk_t[:], bk[:])

                # Prefetch first token tiles of x while W streams in.
                # Each x tile holds a full [H, 128-token] slab as [p, k, t].
                x_tiles = {}

                def load_x(t, r=r):
                    x3 = xpool.tile([P, KT, P], f32r, tag="x", name=f"x3_{r}_{t}")
                    src = xt[:, t * P:(t + 1) * P].rearrange(
                        "(k p) t -> p k t", p=P)
                    nc.sync.dma_start(x3[:], src)
                    x_tiles[t] = x3

                load_x(0)
                load_x(1)

                # Phase 1: stream W in, fold LoRA: W_eff[k] = W[k] + A_k @ B.
                # The DVE add reads raw W (fp32) + lora product (PSUM fp32)
                # and writes the resident W_eff tile as f32r (rounds on
                # output).
                w_tiles = []
                for k in range(KT):
                    wr = wraw.tile([P, N], f32, tag="wr", name=f"wr_{r}_{k}")
                    nc.sync.dma_start(wr[:], w[k * P:(k + 1) * P, :])
                    wt = wpool.tile([P, N], f32r, tag=f"w{k}",
                                    name=f"weff_{r}_{k}")
                    at_t = apool.tile([RANK, P], f32r, tag="at",
                                      name=f"at_{r}_{k}")
                    nc.sync.dma_start(at_t[:], at[:, k * P:(k + 1) * P])
                    for c in range(NCH):
                        ps = psum.tile([P, CH], f32, tag="ps",
                                       name=f"psl_{r}_{k}_{c}")
                        nc.tensor.matmul(
                            ps[:],
                            at_t[:],
                            bk_t[:, c * CH:(c + 1) * CH],
                            start=True, stop=True,
                        )
                        nc.vector.tensor_add(
                            wt[:, c * CH:(c + 1) * CH],
                            wr[:, c * CH:(c + 1) * CH],
                            ps[:],
                        )
                    w_tiles.append(wt)

                # Phase 2: main matmul, token tile by token tile
                for t in range(TT):
                    x3 = x_tiles.pop(t)
                    if t + 2 < TT:
                        load_x(t + 2)
                    pss = [
                        psum.tile([P, CH], f32, tag="ps", name=f"ps_{r}_{t}_{c}")
                        for c in range(NCH)
                    ]
                    for k in range(KT):
                        lhsT = x3[:, k, :]
                        for c in range(NCH):
                            nc.tensor.matmul(
                                pss[c][:],
                                lhsT,
                                w_tiles[k][:, c * CH:(c + 1) * CH],
                                start=(k == 0), stop=(k == KT - 1),
                            )
                    ot = opool.tile([P, N], f32, tag="o", name=f"o_{r}_{t}")
                    for c in range(NCH):
                        nc.vector.tensor_copy(ot[:, c * CH:(c + 1) * CH],
                                              pss[c][:])
                    nc.sync.dma_start(out[t * P:(t + 1) * P, :], ot[:])

    nc.compile()
    return nc


def _prep_inputs(x, W_orig, A_kernel, B_kernel):
    x = np.asarray(x, dtype=np.float32)
    W_orig = np.asarray(W_orig, dtype=np.float32)
    A_kernel = np.asarray(A_kernel, dtype=np.float32)
    B_kernel = np.asarray(B_kernel, dtype=np.float32)

    xT = np.ascontiguousarray(x.reshape(TOK, H).T)          # [H, TOK]
    w2d = np.ascontiguousarray(W_orig.reshape(H, N))        # [H, N]
    at = np.ascontiguousarray(A_kernel.T) * np.float32(SCALE)  # [RANK, H]
    bk = np.ascontiguousarray(B_kernel.reshape(RANK, N))    # [RANK, N]

    in_maps = []
    for i in range(NCORES):
        in_maps.append({
            "xt": np.ascontiguousarray(xT[:, i * TPC:(i + 1) * TPC]),
            "w": w2d,
            "at": at,
            "bk": bk,
        })
    return in_maps


def kernel(x, W_orig, A_kernel, B_kernel):
    from concourse.bass_utils import run_bass_kernel_spmd

    if "nc" not in _CACHE:
        _CACHE["nc"] = _build_program()
    nc = _CACHE["nc"]

    in_maps = _prep_inputs(x, W_orig, A_kernel, B_kernel)
    res = run_bass_kernel_spmd(nc, in_maps, list(range(NCORES)))
    parts = [res.results[i]["out"] for i in range(NCORES)]
    full = np.concatenate(parts, axis=0)                    # [TOK, N]
    return full.reshape(B, S, NH, HD)


# revision 9
# speedup vs baseline: 147.7940x; 1.6750x over previous
"""LoRA layer kernel for Trainium2, SPMD across 8 NeuronCores.

Computes: out[b,s,h,d] = x[b,s,:] @ W_orig[:,h,d] + SCALE * (x @ A) @ B[:,h,d]

Strategy (per sharding hint, data-parallel branch):
  - Fold LoRA into the weights ON DEVICE: W_eff = W + (SCALE*A) @ B
    (associativity of matmul makes this exact up to fp rounding, and it
    turns the whole problem into one dense matmul).
  - Shard x over tokens (B*S = 8192 -> 1024 tokens per core); W/A/B replicated.
  - Per core: out_slice[1024, 2048] = xT_slice.T @ W_eff, accumulated over
    16 K-tiles of 128 into 4 PSUM banks of [128, 512].
  - Matmuls run in float32r mode (fp32 bits, FP22 multiply, fp32 accumulate):
    4x faster than true fp32 on the PE at ~1e-4 relative error.

x is fed pre-transposed ([H, tokens] per core) so the contraction dim lands on
SBUF partitions; host-side layout prep only, all FLOPs happen on device.
"""

import numpy as np

# Problem shapes (hardcoded per contract - kernel.py must be self-contained)
B, S, H = 4, 2048, 2048
NH, HD = 16, 128
N = NH * HD            # 2048 output features
RANK = 4
ALPHA = 4.0
SCALE = ALPHA / RANK   # 1.0
NCORES = 8
TOK = B * S            # 8192 tokens total
TPC = TOK // NCORES    # 1024 tokens per core

P = 128                # SBUF partitions
KT = H // P            # 16 contraction tiles
TT = TPC // P          # 8 token tiles per core
CH = 512               # psum chunk width (fp32 moving-operand / bank limit)
NCH = N // CH          # 4 chunks

_CACHE = {}


def _build_program(reps=1):
    """Build the SPMD program. reps>1 repeats the whole body back-to-back
    (used only for timing: wall(R) - wall(1) cancels host/tunnel overhead)."""
    import concourse.mybir as mybir
    import concourse.tile as tile
    from concourse import bacc

    f32 = mybir.dt.float32
    f32r = mybir.dt.float32r
    bf16 = mybir.dt.bfloat16

    nc = bacc.Bacc(None, target_bir_lowering=False, debug=False)

    # Main matmul runs in bf16 (inputs rounded on device; fp32 PSUM
    # accumulation). The LoRA A@B fold runs in float32r (fp32 bits, FP22
    # multiply) and its DVE add writes the resident W_eff tiles as bf16.
    xt = nc.dram_tensor("xt", [H, TPC], f32, kind="ExternalInput")
    w = nc.dram_tensor("w", [H, N], f32, kind="ExternalInput")
    at = nc.dram_tensor("at", [RANK, H], f32r, kind="ExternalInput")
    bk = nc.dram_tensor("bk", [RANK, N], f32r, kind="ExternalInput")
    out = nc.dram_tensor("out", [TPC, N], f32, kind="ExternalOutput")

    with tile.TileContext(nc) as tc:
        with (
            tc.tile_pool(name="wpool", bufs=1) as wpool,
            tc.tile_pool(name="wraw", bufs=3) as wraw,
            tc.tile_pool(name="xpool", bufs=3) as xpool,
            tc.tile_pool(name="opool", bufs=2) as opool,
            tc.tile_pool(name="cpool", bufs=1) as cpool,
            tc.tile_pool(name="apool", bufs=2) as apool,
            tc.tile_pool(name="psum", bufs=8, space="PSUM") as psum,
        ):
            for r in range(reps):
                # LoRA B matrix, resident: [RANK, N]
                bk_t = cpool.tile([RANK, N], f32r, tag="bk", name=f"bk_{r}")
                nc.sync.dma_start(bk_t[:], bk[:])

                # Prefetch first token tiles of x while W streams in.
                # Each x tile holds a full [H, 128-token] slab as [p, k, t];
                # ScalarE (otherwise idle) downcasts fp32 -> bf16.
                x_tiles = {}

                def load_x(t, r=r):
                    xr = xpool.tile([P, KT, P], f32, tag="xr",
                                    name=f"xr_{r}_{t}")
                    src = xt[:, t * P:(t + 1) * P].rearrange(
                        "(k p) t -> p k t", p=P)
                    nc.sync.dma_start(xr[:], src)
                    x3 = xpool.tile([P, KT, P], bf16, tag="x",
                                    name=f"x3_{r}_{t}")
                    nc.scalar.copy(x3[:], xr[:])
                    x_tiles[t] = x3

                load_x(0)
                load_x(1)

                # Phase 1: stream W in, fold LoRA: W_eff[k] = W[k] + A_k @ B.
                # The DVE add reads raw W (fp32) + lora product (PSUM fp32)
                # and writes the resident W_eff tile as f32r (rounds on
                # output).
                w_tiles = []
                for k in range(KT):
                    wr = wraw.tile([P, N], f32, tag="wr", name=f"wr_{r}_{k}")
                    nc.sync.dma_start(wr[:], w[k * P:(k + 1) * P, :])
                    wt = wpool.tile([P, N], bf16, tag=f"w{k}",
                                    name=f"weff_{r}_{k}")
                    at_t = apool.tile([RANK, P], f32r, tag="at",
                                      name=f"at_{r}_{k}")
                    nc.sync.dma_start(at_t[:], at[:, k * P:(k + 1) * P])
                    for c in range(NCH):
                        ps = psum.tile([P, CH], f32, tag="ps",
                                       name=f"psl_{r}_{k}_{c}")
                        nc.tensor.matmul(
                            ps[:],
                            at_t[:],
                            bk_t[:, c * CH:(c + 1) * CH],
                            start=True, stop=True,
                        )
                        nc.vector.tensor_add(
                            wt[:, c * CH:(c + 1) * CH],
                            wr[:, c * CH:(c + 1) * CH],
                            ps[:],
                        )
                    w_tiles.append(wt)

                # Phase 2: main matmul, token tile by token tile
                for t in range(TT):
                    x3 = x_tiles.pop(t)
                    if t + 2 < TT:
                        load_x(t + 2)
                    pss = [
                        psum.tile([P, CH], f32, tag="ps", name=f"ps_{r}_{t}_{c}")
                        for c in range(NCH)
                    ]
                    for k in range(KT):
                        lhsT = x3[:, k, :]
                        for c in range(NCH):
                            nc.tensor.matmul(
                                pss[c][:],
                                lhsT,
                                w_tiles[k][:, c * CH:(c + 1) * CH],
                                start=(k == 0), stop=(k == KT - 1),
                            )
                    ot = opool.tile([P, N], f32, tag="o", name=f"o_{r}_{t}")
                    for c in range(NCH):
                        nc.vector.tensor_copy(ot[:, c * CH:(c + 1) * CH],
                                              pss[c][:])
                    nc.sync.dma_start(out[t * P:(t + 1) * P, :], ot[:])

    nc.compile()
    return nc


def _prep_inputs(x, W_orig, A_kernel, B_kernel):
    x = np.asarray(x, dtype=np.float32)
    W_orig = np.asarray(W_orig, dtype=np.float32)
    A_kernel = np.asarray(A_kernel, dtype=np.float32)
    B_kernel = np.asarray(B_kernel, dtype=np.float32)

    xT = np.ascontiguousarray(x.reshape(TOK, H).T)          # [H, TOK]
    w2d = np.ascontiguousarray(W_orig.reshape(H, N))        # [H, N]
    at = np.ascontiguousarray(A_kernel.T) * np.float32(SCALE)  # [RANK, H]
    bk = np.ascontiguousarray(B_kernel.reshape(RANK, N))    # [RANK, N]

    in_maps = []
    for i in range(NCORES):
        in_maps.append({
            "xt": np.ascontiguousarray(xT[:, i * TPC:(i + 1) * TPC]),
            "w": w2d,
            "at": at,
            "bk": bk,
        })
    return in_maps


def kernel(x, W_orig, A_kernel, B_kernel):
    from concourse.bass_utils import run_bass_kernel_spmd

    if "nc" not in _CACHE:
        _CACHE["nc"] = _build_program()
    nc = _CACHE["nc"]

    in_maps = _prep_inputs(x, W_orig, A_kernel, B_kernel)
    res = run_bass_kernel_spmd(nc, in_maps, list(range(NCORES)))
    parts = [res.results[i]["out"] for i in range(NCORES)]
    full = np.concatenate(parts, axis=0)                    # [TOK, N]
    return full.reshape(B, S, NH, HD)
